# revision 8
# baseline (speedup 1.0000x reference)
"""AttentiveMatch kernel for Trainium2 (8 NeuronCores, data-parallel over batch).

Reference math (per batch):
    pn = l2norm(p); qn = l2norm(q)
    w  = -(pn @ qn^T) / D          # [S,S]
    mv = (w @ q) / S               # [S,D]
    mn = l2norm(mv)
    out = -mean(pn * mn, -1)       # [S]

Rewritten with u_i = sum_j cos_ij q_j (signs cancel):
    out_i = (1/D) (pn_i . u_i) / |u_i|
    pn_i . u_i = sum_j |q_j| cos_ij^2
    u_i = sum_j qn[j,:] * (|q_j| cos_ij)

Device pipeline (fp8 DoubleRow matmuls, fp32 PSUM accumulation):
    C'   = (s1 qn)^T-layout @ (s1 pn)^T-layout        [S,S] per j-tile (mm1)
    A8   = fp8(g |q_j| cos)      ACT copy w/ per-partition scale from PSUM
    h    = |q_j| cos^2 (bf16)    ACT square w/ per-partition scale from PSUM
    dot row  = ones^T @ (h0+h1+h2+h3)                 [1,S]
    M'   = (s1 qn natural) @ A8                       [D,S] (mm2), = s1 g u
    s2   = M'^2 (bf16, ACT square), ss row = ones^T @ sum(s2)
Rows are DMA'd out raw; the final out = (s1 g / D) dot / sqrt(ss) runs on host.

Each core handles 8 batches; norms/normalization precomputed on host.
"""

import os
import sys

for _p in ("/opt/trn_rl_repo",):
    if _p not in sys.path:
        sys.path.append(_p)

import numpy as np
import ml_dtypes

import concourse.bacc as bacc
import concourse.mybir as mybir
import concourse.tile as tile
from concourse.bass_utils import run_bass_kernel_spmd

B, S, D = 64, 512, 768
NCORES = 8
BP = B // NCORES          # batches per core
ST = S // 128             # s tiles (4)
KT = D // 128             # d tiles (6)
F32 = mybir.dt.float32
BF16 = mybir.dt.bfloat16
F8 = mybir.dt.float8e4
AF = mybir.ActivationFunctionType
DR = mybir.MatmulPerfMode.DoubleRow

S1 = 16.0                 # fp8 input pre-scale
G = 16.0                  # fp8 A-matrix pre-scale

_NC = None

if os.environ.get("KERNEL_LDW_OPT", "0") == "1":
    import concourse.bass_utils as _bu

    _orig_run_command = _bu.run_command

    def _patched_run_command(cmd, **kw):
        cmd = [
            ("--enable-ldw-opt=true" if c == "--enable-ldw-opt=false" else c)
            for c in cmd
        ]
        return _orig_run_command(cmd, **kw)

    _bu.run_command = _patched_run_command


def _build():
    nc = bacc.Bacc("TRN2", target_bir_lowering=False, debug=False, num_devices=NCORES)
    qt_d = nc.dram_tensor("qt", [BP, 128, KT, S], F8, kind="ExternalInput")
    pt_d = nc.dram_tensor("pt", [BP, 128, KT, S], F8, kind="ExternalInput")
    qn_d = nc.dram_tensor("qn", [BP, 128, ST, D], F8, kind="ExternalInput")
    sc_d = nc.dram_tensor("sc", [BP, 128, 2 * ST], F32, kind="ExternalInput")
    out_d = nc.dram_tensor("out", [1, 2 * BP * S], F32, kind="ExternalOutput")

    with tile.TileContext(nc) as tc:
        with (
            tc.tile_pool(name="cst", bufs=1) as cst,
            tc.tile_pool(name="inp", bufs=3) as inp,
            tc.tile_pool(name="ats", bufs=2) as ats,
            tc.tile_pool(name="gps", bufs=2, space="PSUM") as gps,
            tc.tile_pool(name="mps", bufs=3, space="PSUM") as mps,
            tc.tile_pool(name="rps", bufs=2, space="PSUM") as rps,
            tc.tile_pool(name="scr", bufs=2) as scr,
        ):
            ones16 = cst.tile([128, 1], BF16)
            nc.gpsimd.memset(ones16[:], 1.0)
            out_sb = cst.tile([1, 2 * BP * S], F32)

            pend = None  # deferred ss-reduce of the previous batch
            for b in range(BP):
                # qt on sync ring, pt on scalar ring, in 2-ktile chunks that
                # match the DoubleRow accumulation steps; qn on gpsimd ring.
                sc_t = inp.tile([128, 2 * ST], F32, tag="sc")
                nc.scalar.dma_start(sc_t[:], sc_d[b])
                qt = inp.tile([128, KT, S], F8, tag="qt")
                pt = inp.tile([128, KT, S], F8, tag="pt")
                for c in range(KT // 2):
                    sl = slice(2 * c, 2 * c + 2)
                    nc.sync.dma_start(qt[:, sl, :], qt_d[b, :, sl, :])
                    nc.scalar.dma_start(pt[:, sl, :], pt_d[b, :, sl, :])
                qn = inp.tile([128, ST, D], F8, tag="qn")
                nc.gpsimd.dma_start(qn[:], qn_d[b])

                # mm1: C'[j,i] = sum_d (s1 qn)[j,d] (s1 pn)[i,d], 3 DoubleRow
                # steps per j-tile; A8 and h peeled off each PSUM tile.
                at_all = ats.tile([128, ST, S], F8, tag="at")
                h_t = []
                for j in range(ST):
                    g_ps = gps.tile([128, S], F32, tag="g")
                    for c in range(KT // 2):
                        nc.tensor.matmul(
                            g_ps[:],
                            lhsT=qt[:, 2 * c:2 * c + 2, j * 128:(j + 1) * 128],
                            rhs=pt[:, 2 * c:2 * c + 2, :],
                            start=(c == 0), stop=(c == KT // 2 - 1),
                            perf_mode=DR,
                        )
                    nc.scalar.activation(at_all[:, j, :], g_ps[:], AF.Copy,
                                         scale=sc_t[:, j:j + 1])
                    h = scr.tile([128, S], BF16, tag=f"h{j}")
                    nc.scalar.activation(h[:], g_ps[:], AF.Square,
                                         scale=sc_t[:, ST + j:ST + j + 1])
                    h_t.append(h)

                # deferred ss-reduce from previous batch (its s2 adds are
                # long done by now, so the PE never stalls on them)
                if pend is not None:
                    s2s_p, rows_p, bp = pend
                    nc.tensor.matmul(rows_p[32:33, :], lhsT=ones16[:],
                                     rhs=s2s_p[:], start=True, stop=True)
                    nc.vector.tensor_copy(
                        out_sb[0:1, (BP + bp) * S:(BP + bp + 1) * S],
                        rows_p[32:33, :])

                rows = rps.tile([64, S], F32, tag="rows")

                # dot-row pre-adds on DVE while mm2 runs
                h01 = scr.tile([128, S], BF16, tag="h01")
                nc.vector.tensor_add(h01[:], h_t[0][:], h_t[1][:])
                h23 = scr.tile([128, S], BF16, tag="h23")
                nc.vector.tensor_add(h23[:], h_t[2][:], h_t[3][:])
                hs = scr.tile([128, S], BF16, tag="hs")
                nc.vector.tensor_add(hs[:], h01[:], h23[:])

                # mm2: M'[d,i] = sum_j (s1 qn)[j,d] A8[j,i], 2 DoubleRow steps
                # per d-tile; s2 = M'^2 peeled off each PSUM tile.
                s2_t = []
                for k in range(KT):
                    m_ps = mps.tile([128, S], F32, tag="m")
                    for c in range(ST // 2):
                        nc.tensor.matmul(
                            m_ps[:],
                            lhsT=qn[:, 2 * c:2 * c + 2, k * 128:(k + 1) * 128],
                            rhs=at_all[:, 2 * c:2 * c + 2, :],
                            start=(c == 0), stop=(c == ST // 2 - 1),
                            perf_mode=DR,
                        )
                    s2 = scr.tile([128, S], BF16, tag=f"s2_{k}")
                    nc.scalar.activation(s2[:], m_ps[:], AF.Square)
                    s2_t.append(s2)

                # dot-row reduce (hs ready well before mm2 finishes)
                nc.tensor.matmul(rows[0:1, :], lhsT=ones16[:], rhs=hs[:],
                                 start=True, stop=True)
                nc.vector.tensor_copy(out_sb[0:1, b * S:(b + 1) * S], rows[0:1, :])

                # ss pre-adds on DVE; the final reduce is deferred past the
                # next batch's mm1 so the PE queue never waits on them
                sa = scr.tile([128, S], BF16, tag="sa")
                nc.vector.tensor_add(sa[:], s2_t[0][:], s2_t[1][:])
                sb = scr.tile([128, S], BF16, tag="sb")
                nc.vector.tensor_add(sb[:], s2_t[2][:], s2_t[3][:])
                sc2 = scr.tile([128, S], BF16, tag="sc2")
                nc.vector.tensor_add(sc2[:], s2_t[4][:], s2_t[5][:])
                sab = scr.tile([128, S], BF16, tag="sab")
                nc.vector.tensor_add(sab[:], sa[:], sb[:])
                s2s = scr.tile([128, S], BF16, tag="s2s")
                nc.vector.tensor_add(s2s[:], sab[:], sc2[:])
                pend = (s2s, rows, b)

            s2s_p, rows_p, bp = pend
            nc.tensor.matmul(rows_p[32:33, :], lhsT=ones16[:], rhs=s2s_p[:],
                             start=True, stop=True)
            nc.vector.tensor_copy(out_sb[0:1, (BP + bp) * S:(BP + bp + 1) * S],
                                  rows_p[32:33, :])
            nc.sync.dma_start(out_d[:], out_sb[:])
    nc.compile()
    return nc


def _get_nc():
    global _NC
    if _NC is None:
        _NC = _build()
    return _NC


def _prep_inputs(p, q):
    p = np.asarray(p, dtype=np.float32)
    q = np.asarray(q, dtype=np.float32)
    nq = np.sqrt(np.maximum((q * q).sum(-1), 1e-12))          # [B,S]
    npn = np.sqrt(np.maximum((p * p).sum(-1), 1e-12))
    qn8 = (q * (S1 / nq[..., None])).astype(ml_dtypes.float8_e4m3)
    pn8 = (p * (S1 / npn[..., None])).astype(ml_dtypes.float8_e4m3)

    # natural: [core, b, part, jt, d] with s = jt*128 + part
    qn_nat = np.ascontiguousarray(
        qn8.reshape(NCORES, BP, ST, 128, D).transpose(0, 1, 3, 2, 4)
    )
    # transposed: [core, b, part, kt, s] with d = kt*128 + part
    def tr(x):
        return np.ascontiguousarray(
            x.reshape(NCORES, BP, S, KT, 128).transpose(0, 1, 4, 3, 2)
        )

    qt, pt = tr(qn8), tr(pn8)
    # per-j scales, [core, b, part, jt]: A-scale then h-scale
    nqr = nq.reshape(NCORES, BP, ST, 128).transpose(0, 1, 3, 2)
    sc = np.concatenate(
        [nqr * (G / (S1 * S1)), np.sqrt(nqr) * (1.0 / (S1 * S1))], axis=-1
    ).astype(np.float32)
    return [
        {"qt": qt[c], "pt": pt[c], "qn": qn_nat[c], "sc": sc[c]}
        for c in range(NCORES)
    ]


def _postprocess(results):
    o = np.stack([np.asarray(r["out"], dtype=np.float32) for r in results])
    o = o.reshape(NCORES, 2 * BP, S)
    dot = o[:, :BP, :].reshape(B, S)
    ssr = np.maximum(o[:, BP:, :].reshape(B, S), 1e-20)
    wd = (S1 * G / D) * dot / np.sqrt(ssr)
    return np.ascontiguousarray(wd[:, None, :])


def _run(inputs, trace=False, **kw):
    nc = _get_nc()
    in_maps = _prep_inputs(inputs["p"], inputs["q"])
    res = run_bass_kernel_spmd(nc, in_maps, list(range(NCORES)), trace=trace, **kw)
    return _postprocess(res.results), res


def kernel(p, q):
    out, _ = _run({"p": p, "q": q})
    return out


# revision 9
# speedup vs baseline: 1.9367x; 1.9367x over previous
"""AttentiveMatch kernel for Trainium2 (8 NeuronCores, data-parallel over batch).

Reference math (per batch):
    pn = l2norm(p); qn = l2norm(q)
    w  = -(pn @ qn^T) / D          # [S,S]
    mv = (w @ q) / S               # [S,D]
    mn = l2norm(mv)
    out = -mean(pn * mn, -1)       # [S]

Rewritten with u_i = sum_j cos_ij q_j (signs cancel):
    out_i = (1/D) (pn_i . u_i) / |u_i|
    pn_i . u_i = sum_j |q_j| cos_ij^2
    u_i = sum_j qn[j,:] * (|q_j| cos_ij)

Device pipeline (fp8 DoubleRow matmuls, all scalar folding done on host):
    mm1 [j,i]:  C' = (s2 q)^T-layout.T @ (s1 pn)^T-layout = s1 s2 |q_j| cos
    A8 = fp8(C' * G/(s1 s2))            DVE tensor_scalar, const scale
    h  = Square(C' / (s1 s2))           ACT, bf16: (|q_j| cos)^2
    dot row = sum_jt (1/|q_j|)^T @ h    4 weighted PE reduce matmuls
    mm2 [i,d]:  M2 = A8-lhsT.T @ (s1 qn) = s1 G u   (i-partition layout)
    ss col = accum_out of Square(M2/256) ACT, free-axis reduce, final layout
Host: out = dot / (D sqrt(ss)).  Norms |p|,|q| precomputed on host.
"""

import os
import sys

for _p in ("/opt/trn_rl_repo",):
    if _p not in sys.path:
        sys.path.append(_p)

import numpy as np
import ml_dtypes

import concourse.bacc as bacc
import concourse.mybir as mybir
import concourse.tile as tile
from concourse.bass_utils import run_bass_kernel_spmd

B, S, D = 64, 512, 768
NCORES = 8
BP = B // NCORES          # batches per core
ST = S // 128             # s tiles (4)
KT = D // 128             # d tiles (6)
F32 = mybir.dt.float32
BF16 = mybir.dt.bfloat16
F8 = mybir.dt.float8e4
AF = mybir.ActivationFunctionType
DR = mybir.MatmulPerfMode.DoubleRow

S1 = 16.0                 # fp8 pre-scale for pn/qn operands
S2 = 8.0                  # fp8 pre-scale for the raw-q operand
G = 16.0                  # fp8 A-matrix scale; S1*G = 256 so ss = |u|^2
C_A = G / (S1 * S2)       # PSUM -> A8
C_H = 1.0 / (S1 * S2)     # PSUM -> h
C_S = 1.0 / (S1 * G)      # M2 -> u

_NC = None

if os.environ.get("KERNEL_LDW_OPT", "0") == "1":
    import concourse.bass_utils as _bu

    _orig_run_command = _bu.run_command

    def _patched_run_command(cmd, **kw):
        cmd = [
            ("--enable-ldw-opt=true" if c == "--enable-ldw-opt=false" else c)
            for c in cmd
        ]
        return _orig_run_command(cmd, **kw)

    _bu.run_command = _patched_run_command


def _build():
    nc = bacc.Bacc("TRN2", target_bir_lowering=False, debug=False, num_devices=NCORES)
    qt_d = nc.dram_tensor("qt", [BP, 128, KT, S], F8, kind="ExternalInput")
    pt_d = nc.dram_tensor("pt", [BP, 128, KT, S], F8, kind="ExternalInput")
    qn_d = nc.dram_tensor("qn", [BP, 128, ST, D], F8, kind="ExternalInput")
    w_d = nc.dram_tensor("w", [BP, 128, ST], BF16, kind="ExternalInput")
    od_d = nc.dram_tensor("od", [1, BP * S], F32, kind="ExternalOutput")
    os_d = nc.dram_tensor("os", [128, BP * ST], F32, kind="ExternalOutput")

    with tile.TileContext(nc) as tc:
        with (
            tc.tile_pool(name="cst", bufs=1) as cst,
            tc.tile_pool(name="inp", bufs=3) as inp,
            tc.tile_pool(name="ats", bufs=2) as ats,
            tc.tile_pool(name="gps", bufs=3, space="PSUM") as gps,
            tc.tile_pool(name="mps", bufs=2, space="PSUM") as mps,
            tc.tile_pool(name="rps", bufs=1, space="PSUM") as rps,
            tc.tile_pool(name="scr", bufs=2) as scr,
        ):
            out_dot = cst.tile([1, BP * S], F32)
            out_ss = cst.tile([128, BP * ST], F32)

            for b in range(BP):
                w16 = inp.tile([128, ST], BF16, tag="w")
                nc.sync.dma_start(w16[:], w_d[b])
                qt = inp.tile([128, KT, S], F8, tag="qt")
                pt = inp.tile([128, KT, S], F8, tag="pt")
                for c in range(KT // 2):
                    sl = slice(2 * c, 2 * c + 2)
                    nc.sync.dma_start(qt[:, sl, :], qt_d[b, :, sl, :])
                    nc.gpsimd.dma_start(pt[:, sl, :], pt_d[b, :, sl, :])
                qn = inp.tile([128, ST, D], F8, tag="qn")
                nc.gpsimd.dma_start(qn[:], qn_d[b])

                # mm1: C'[j,i] = sum_d (s2 q)[j,d] (s1 pn)[i,d]; A8 peel on
                # DVE (const scale), h = (|q| cos)^2 peel on ACT.
                at_all = ats.tile([128, ST, S], F8, tag="at")
                h_t = []
                for j in range(ST):
                    g_ps = gps.tile([128, S], F32, tag="g")
                    for c in range(KT // 2):
                        nc.tensor.matmul(
                            g_ps[:],
                            lhsT=qt[:, 2 * c:2 * c + 2, j * 128:(j + 1) * 128],
                            rhs=pt[:, 2 * c:2 * c + 2, :],
                            start=(c == 0), stop=(c == KT // 2 - 1),
                            perf_mode=DR,
                        )
                    nc.vector.tensor_scalar_mul(at_all[:, j, :], g_ps[:], C_A)
                    h = scr.tile([128, S], BF16, tag=f"h{j}")
                    nc.scalar.activation(h[:], g_ps[:], AF.Square, scale=C_H)
                    h_t.append(h)

                # mm2 (i-partition layout): M2[i,d] = sum_j A8[j,i] (s1 qn)[j,d]
                # d split 512+256 to respect the one-bank matmul-out rule;
                # ss_i = sum_d (M2/256)^2 via ACT Square accum_out -> final
                # [i%128, it] layout with no further reduction.
                sq = scr.tile([128, D], BF16, tag="sq")
                for it in range(ST):
                    m2 = mps.tile([128, 1024], F32, tag="m2")
                    lh = [at_all[:, 2 * c:2 * c + 2, it * 128:(it + 1) * 128]
                          for c in range(ST // 2)]
                    for lo, hi in ((0, 512), (512, 768)):
                        for c in range(ST // 2):
                            nc.tensor.matmul(
                                m2[:, lo:hi],
                                lhsT=lh[c],
                                rhs=qn[:, 2 * c:2 * c + 2, lo:hi],
                                start=(c == 0), stop=(c == ST // 2 - 1),
                                perf_mode=DR,
                            )
                    nc.scalar.activation(
                        sq[:], m2[:, 0:D], AF.Square, scale=C_S,
                        accum_out=out_ss[:, b * ST + it:b * ST + it + 1])

                # dot row: 4 weighted partition-reduces on the PE (h ready
                # long before mm2 finishes)
                rows = rps.tile([1, S], F32, tag="rows")
                for jt in range(ST):
                    nc.tensor.matmul(rows[0:1, :], lhsT=w16[:, jt:jt + 1],
                                     rhs=h_t[jt][:],
                                     start=(jt == 0), stop=(jt == ST - 1))
                nc.vector.tensor_copy(out_dot[0:1, b * S:(b + 1) * S],
                                      rows[0:1, :])

            nc.sync.dma_start(od_d[:], out_dot[:])
            nc.sync.dma_start(os_d[:], out_ss[:])
    nc.compile()
    return nc


def _get_nc():
    global _NC
    if _NC is None:
        _NC = _build()
    return _NC


def _prep_inputs(p, q):
    p = np.asarray(p, dtype=np.float32)
    q = np.asarray(q, dtype=np.float32)
    nq = np.sqrt(np.maximum((q * q).sum(-1), 1e-12))          # [B,S]
    npn = np.sqrt(np.maximum((p * p).sum(-1), 1e-12))
    q8 = (q * S2).astype(ml_dtypes.float8_e4m3)               # s2 * q
    pn8 = (p * (S1 / npn[..., None])).astype(ml_dtypes.float8_e4m3)
    qn8 = (q * (S1 / nq[..., None])).astype(ml_dtypes.float8_e4m3)

    # transposed: [core, b, part, kt, s] with d = kt*128 + part
    def tr(x):
        return np.ascontiguousarray(
            x.reshape(NCORES, BP, S, KT, 128).transpose(0, 1, 4, 3, 2)
        )

    # natural: [core, b, part, jt, d] with s = jt*128 + part
    qn_nat = np.ascontiguousarray(
        qn8.reshape(NCORES, BP, ST, 128, D).transpose(0, 1, 3, 2, 4)
    )
    qt, pt = tr(q8), tr(pn8)
    w = np.ascontiguousarray(
        (1.0 / nq).reshape(NCORES, BP, ST, 128).transpose(0, 1, 3, 2)
    ).astype(ml_dtypes.bfloat16)
    return [
        {"qt": qt[c], "pt": pt[c], "qn": qn_nat[c], "w": w[c]}
        for c in range(NCORES)
    ]


def _postprocess(results):
    dot = np.stack([np.asarray(r["od"], dtype=np.float32) for r in results])
    dot = dot.reshape(B, S)
    ss = np.stack([np.asarray(r["os"], dtype=np.float32) for r in results])
    # os[core, part, b*ST + it] is |u|^2 for batch core*BP+b at i = it*128+part
    ss = ss.reshape(NCORES, 128, BP, ST).transpose(0, 2, 3, 1).reshape(B, S)
    wd = dot / (D * np.sqrt(np.maximum(ss, 1e-20)))
    return np.ascontiguousarray(wd[:, None, :])


def _run(inputs, trace=False, **kw):
    nc = _get_nc()
    in_maps = _prep_inputs(inputs["p"], inputs["q"])
    res = run_bass_kernel_spmd(nc, in_maps, list(range(NCORES)), trace=trace, **kw)
    return _postprocess(res.results), res


def kernel(p, q):
    out, _ = _run({"p": p, "q": q})
    return out
